# revision 26
# baseline (speedup 1.0000x reference)
"""Trainium2 Bass kernel for nn_BlockSelfAttentionModule (v5, 19.2us HW).

Math: out[b*H+h, l, m] = sum_d q[b*H+h, l, d] * R_h[l, m, d]
  R_h[l, m, :] = r_voice[l%8, m%8, :, h]
               + (e_past[fi-fj, :, h] if fj <= fi else e_future[fj-fi, :, h])
  with fi = l//8, fj = m//8.  out[l, m] = T[l, m//8] + V[l, m%8].

Layout (per core = head h): partition p = 32*u + 4*di + b; l-tiles t = 0..11
cover frames fi = 4t + u, l = 32t + 8u + di.  Design points (vs the 28.5us
v1 kernel):
 - q arrives HOST-PRETRANSPOSED as the matmul lhsT, CONCATENATED with the
   block-diagonal weight matrix into ONE bf16 dram tensor: a single load DMA
   gates all four matmuls (v1: transposes on PE + multiple fp32 loads).
   NOTE: the host MUST convert to ml_dtypes.bfloat16 -- feeding f32 numpy
   into a bf16 dram tensor silently reinterprets bytes on HW (NaNs).
 - 4 bf16 matmuls (full PE rate), N = 348 = 3 tau-blocks of
   [51 G-window | 64 U | 1 pad], one PSUM bank per g (bank-aligned).
 - the skew shift u is constant per 32-partition QUAD, so the time gather is
   16 plain 3-D copies at legal partition bases 0/32/64/96 on DVE/ACT
   reading PSUM directly (v1 burned the shared HWDGE ring on ~20 tiny DMAs;
   GPSIMD cannot touch PSUM, so Pool gets only SBUF work).
 - voice: U cols staged PSUM->SBUF (4 ACT copies), then 8 copy_predicated
   (per di) with mask mk[p, di] = ((p//4)%8 == di).  The mask is loaded and
   staged BEFORE the loop (u8 -> f32 on ACT -> u8 on DVE); the ACT op in
   that chain anchors the activation-table load on the loop-entry path,
   keeping 1.28us of LoadActFuncSet out of the For_i body.
 - output path is bf16 (tolerance 2e-2 >> measured 6.5e-3): halves store
   bytes and enables the DVE 2x 16-bit mode for the packed "+= vt" pass.
 - final add osb[p, tau*384+fj*8+r] = ts[p, tau*48+47-fj] + vt[..r] per
   tile: ACT/Pool broadcast-expand + DVE packed-2x add, or Pool/DVE fused
   tensor_add (ACT cannot tensor_add).
 - APs on shared tensors stay <= 3 dims and hot tensors are bufs=1:
   multi-buf tiles and 4-D APs degrade the dep tracker to whole-tensor-
   write, serializing every accessor pair (4-D is used only where the
   implied deps are real anyway: voice over usb/vt).
 - ts/osb are split per g so WAR deps retire per-tile in the loop.
 - out DRAM is PARTITION-MAJOR [t, p, m] so stores are 4 fully-contiguous
   3-tile DMAs (l-major order needs (k8,b,t,m) = 4 AP dims); the host
   un-permutes with one numpy transpose.  Stores split sync/scalar so the
   next iteration's load DMA is not queued behind them on SP's SEQ.
 - For_i(staggered_reset=True): stage-wise semaphore resets instead of a
   per-iteration all-engine barrier + drain (the plain For_i fully
   serializes iterations; with it the HW loop period was 24-27us).

Sharding: head-parallel, core h handles head h (4 batch rows of the output).
"""

import os
import sys

for _p in ("/opt/trn_rl_repo", "/root/.axon_site/_ro/trn_rl_repo"):
    if os.path.isdir(_p) and _p not in sys.path:
        sys.path.insert(0, _p)

import contextlib

import numpy as np

import concourse.bass as bass
import concourse.bacc as bacc
import concourse.mybir as mybir
import concourse.tile as tile
from concourse.bass_utils import run_bass_kernel_spmd

E, H, DI, DO, F = 16, 8, 8, 8, 48
L = F * DI  # 384
B = 4
NG = 51             # G window cols per tau-block
NU = DI * DO        # 64 voice cols
NBLK = NG + NU + 1  # 116 padded block width
NMM = 3 * NBLK      # 348: matmul N (even, >=256 for fp32r full rate)
GSTR = 512          # PSUM bank stride
NW = 4 * NMM        # 1392 W48 cols
NQW = 512 + NW      # 1904: fused [qt | W48] row length
NCORES = 8
DT = mybir.dt.float32
DTR = mybir.dt.float32r
DTH = mybir.dt.bfloat16

_prog_cache = {}

# mk[p, di] = 1 where (p//4) % 8 == di (voice copy_predicated select)
MSK = np.ascontiguousarray(
    ((np.arange(128)[:, None] // 4) % 8 == np.arange(8)[None, :]).astype(np.uint8)
)


def build_program(loop_n=None):
    nc = bacc.Bacc("TRN2", target_bir_lowering=False, debug=False)
    qw = nc.dram_tensor("qw", [48, NQW], DTH, kind="ExternalInput")
    mk = nc.dram_tensor("mk", [128, 8], mybir.dt.uint8, kind="ExternalInput")
    out = nc.dram_tensor("out", [12, 128, L], DTH, kind="ExternalOutput")

    with tile.TileContext(nc) as tc, contextlib.ExitStack() as ctx:
        const_pool = ctx.enter_context(tc.tile_pool(name="const", bufs=1))
        q_pool = ctx.enter_context(tc.tile_pool(name="q", bufs=2))
        zp_pool = ctx.enter_context(tc.tile_pool(name="zp", bufs=1, space="PSUM"))
        wk_pool = ctx.enter_context(tc.tile_pool(name="wk", bufs=1))

        # The voice mask is a kernel constant: load it BEFORE the loop, and
        # stage it through an ACT copy that voice reads.  The copy anchors an
        # activation op on the loop-entry path, which lets the act-table pass
        # keep the 1.28us LoadActFuncSet out of the For_i body.
        msk0 = const_pool.tile([128, 8], mybir.dt.uint8)
        nc.sync.dma_start(msk0[:], mk[:])
        mskf = const_pool.tile([128, 8], DT)
        nc.scalar.copy(mskf[:], msk0[:])     # also anchors the act table load
        msk = const_pool.tile([128, 8], mybir.dt.uint8)
        nc.vector.tensor_copy(msk[:], mskf[:])

        loop_ctx = (tc.For_i(0, loop_n, 1, staggered_reset=True)
                    if loop_n else contextlib.nullcontext())
        ctx.enter_context(loop_ctx)

        qws = q_pool.tile([48, NQW], DTH, tag="qws")
        nc.sync.dma_start(qws[:], qw[:])

        # dep-free warm-up matmul: ramps the PE out of its cold p-state
        # while the qw load is in flight (first real matmul 535 -> 290 ns)
        zw = zp_pool.tile([2, 8], DT, tag="zwarm")
        nc.tensor.matmul(zw[:], mskf[:2, 0:2], mskf[:2, 0:8])

        z_tiles = []
        for g in range(4):
            z_g = zp_pool.tile([128, GSTR], DT, tag=f"z{g}")
            nc.tensor.matmul(
                z_g[:, 0:NMM],
                qws[:, g * 128:(g + 1) * 128],
                qws[:, 512 + g * NMM:512 + (g + 1) * NMM],
            )
            z_tiles.append(z_g)

        # work tiles: ts/osb split per g (fine WAR retirement + precise deps);
        # usb/vt single so voice is 8 ops (their 4-D whole-tensor fallback
        # only creates dependencies that are real anyway)
        usb = wk_pool.tile([128, 768], DTH, tag="usb")
        tsg = [wk_pool.tile([128, 144], DTH, tag=f"ts{g}", name=f"ts{g}")
               for g in range(4)]
        vt = wk_pool.tile([128, 96], DTH, tag="vt")
        osb = [wk_pool.tile([128, 3 * L], DTH, tag=f"osb{j}", name=f"osb{j}")
               for j in range(4)]

        def u_copy(g):
            # usb[p, g*192 + tau*64 + 8di + do]  (ACT, from PSUM)
            u_src = bass.AP(
                z_tiles[g].tensor, NG, [[GSTR, 128], [NBLK, 3], [1, NU]]
            )
            u_dst = bass.AP(
                usb.tensor, g * 192, [[768, 128], [NU, 3], [1, NU]]
            )
            nc.scalar.copy(u_dst, u_src)

        def gather(u, g):
            # ts[g][p, tau*48 + k] = Z[p, tau*116 + u + k], 3-D from PSUM
            src = bass.AP(
                z_tiles[g].tensor, (32 * u) * GSTR + u,
                [[GSTR, 32], [NBLK, 3], [1, F]],
            )
            dst = bass.AP(
                tsg[g].tensor, (32 * u) * 144, [[144, 32], [F, 3], [1, F]]
            )
            if u % 2 == 1:
                nc.scalar.copy(dst, src)
            else:
                nc.vector.tensor_copy(dst, src)

        def voice(di):
            # vt[p, (3g+tau)*8 + r], predicated on di(p) == di (DVE)
            data = bass.AP(
                usb.tensor, 8 * di, [[768, 128], [192, 4], [1, 8], [NU, 3]]
            )
            mask = bass.AP(msk.tensor, di, [[8, 128], [0, 4], [0, 8], [0, 3]])
            vout = bass.AP(vt.tensor, 0, [[96, 128], [24, 4], [1, 8], [8, 3]])
            nc.vector.copy_predicated(vout, mask, data)

        def t_aps(g, tau):
            t_ap = bass.AP(
                tsg[g].tensor, tau * F + 47, [[144, 128], [-1, F], [0, 8]]
            )
            v_ap = bass.AP(
                vt.tensor, (3 * g + tau) * 8, [[96, 128], [0, F], [1, 8]]
            )
            o_ap = bass.AP(
                osb[g].tensor, tau * L, [[3 * L, 128], [8, F], [1, 8]]
            )
            return t_ap, v_ap, o_ap

        def expand(g, tau, eng):
            t_ap, _, o_ap = t_aps(g, tau)
            if eng is nc.scalar:
                nc.scalar.copy(o_ap, t_ap)
            else:
                nc.gpsimd.tensor_copy(o_ap, t_ap)

        def pass2(g, tau):
            _, v_ap, o_ap = t_aps(g, tau)
            nc.vector.tensor_add(o_ap, o_ap, v_ap)  # packed bf16 2x +=

        def fused(g, tau, eng=None):
            t_ap, v_ap, o_ap = t_aps(g, tau)
            (eng or nc.gpsimd).tensor_add(o_ap, t_ap, v_ap)

        def store(j):
            st_src = bass.AP(osb[j].tensor, 0, [[3 * L, 128], [L, 3], [1, L]])
            st_dst = bass.AP(
                out, j * 3 * 128 * L, [[L, 128], [128 * L, 3], [1, L]]
            )
            # j2/j3 on scalar: keeps SP's in-order SEQ free so the next
            # iteration's qw load issues right after j1
            (nc.sync if j < 1 else nc.scalar).dma_start(st_dst, st_src)

        u_copy(0), u_copy(1)
        for g in range(2):
            gather(0, g); gather(2, g)     # DVE
            gather(1, g); gather(3, g)     # ACT
        u_copy(2), u_copy(3)
        for g in range(2, 4):
            gather(0, g); gather(2, g)
            gather(1, g); gather(3, g)
        for di in range(8):
            voice(di)                      # DVE (needs all U)
        for g in range(4):
            expand(g, 0, nc.scalar if g != 2 else nc.gpsimd)
            expand(g, 1, nc.scalar if g == 2 else nc.gpsimd)
            pass2(g, 0)
            pass2(g, 1)
            fused(g, 2, eng=nc.vector if g == 3 else None)
            store(g)

    nc.compile()
    return nc


def _get_program():
    if "nc" not in _prog_cache:
        _prog_cache["nc"] = build_program()
    return _prog_cache["nc"]


def make_core_inputs(q, r_voice, e_past, e_future):
    """Host-side sharding: per-head [pretransposed-q | block-diag W48]."""
    q = np.ascontiguousarray(q, dtype=np.float32)
    qr = q.reshape(B, H, L, E)
    in_maps = []
    for h in range(NCORES):
        qh = qr[:, h]  # (B, L, E)
        # lhsT[16*tau + d, g*128 + p] = q[b, l, d],
        # p = 32u + 4di + b, l = 8*(12g + 4tau + u) + di
        lt = qh.reshape(B, 4, 3, 4, 8, E)      # (b, g, tau, u, di, d)
        lt = lt.transpose(2, 5, 1, 3, 4, 0)    # (tau, d, g, u, di, b)
        qw = np.zeros((48, NQW), dtype=np.float32)
        qw[:, :512] = lt.reshape(48, 4 * 128)
        master = np.zeros((E, 95), dtype=np.float32)
        master[:, :47] = e_future[1:48, :, h][::-1].T
        master[:, 47:] = e_past[:, :, h].T
        U = r_voice[:, :, :, h].reshape(DI * DO, E).T
        for g in range(4):
            for tau in range(3):
                c0 = 512 + g * NMM + tau * NBLK
                F0 = 4 * (3 * g + tau)
                qw[16 * tau:16 * tau + 16, c0:c0 + NG] = master[:, F0:F0 + NG]
                qw[16 * tau:16 * tau + 16, c0 + NG:c0 + NG + NU] = U
        qw16 = np.ascontiguousarray(qw.astype(mybir.dt.np(DTH)))
        in_maps.append({"qw": qw16, "mk": MSK})
    return in_maps


def kernel(q, flipped_masks, r_voice, e_past, e_future):
    q = np.asarray(q, dtype=np.float32)
    r_voice = np.asarray(r_voice, dtype=np.float32)
    e_past = np.asarray(e_past, dtype=np.float32)
    e_future = np.asarray(e_future, dtype=np.float32)

    nc = _get_program()
    in_maps = make_core_inputs(q, r_voice, e_past, e_future)
    res = run_bass_kernel_spmd(nc, in_maps, core_ids=list(range(NCORES)))

    # device out is [t, p, m] with p = 32u + 4di + b; un-permute to l-major
    # (l = 32t + 8u + di) and cast bf16 -> f32 host-side.
    out = np.empty((B * H, L, L), dtype=np.float32)
    for h in range(NCORES):
        arr = np.asarray(res.results[h]["out"], dtype=np.float32)
        arr = arr.reshape(12, 4, 8, B, L).transpose(3, 0, 1, 2, 4)  # (b,t,u,di,m)
        for b in range(B):
            out[b * H + h] = arr[b].reshape(L, L)
    return out


# revision 29
# speedup vs baseline: 1.0190x; 1.0190x over previous
"""Trainium2 Bass kernel for nn_BlockSelfAttentionModule (v5, 19.2us HW).

Math: out[b*H+h, l, m] = sum_d q[b*H+h, l, d] * R_h[l, m, d]
  R_h[l, m, :] = r_voice[l%8, m%8, :, h]
               + (e_past[fi-fj, :, h] if fj <= fi else e_future[fj-fi, :, h])
  with fi = l//8, fj = m//8.  out[l, m] = T[l, m//8] + V[l, m%8].

Layout (per core = head h): partition p = 32*u + 4*di + b; l-tiles t = 0..11
cover frames fi = 4t + u, l = 32t + 8u + di.  Design points (vs the 28.5us
v1 kernel):
 - q arrives HOST-PRETRANSPOSED as the matmul lhsT, CONCATENATED with the
   block-diagonal weight matrix into ONE bf16 dram tensor: a single load DMA
   gates all four matmuls (v1: transposes on PE + multiple fp32 loads).
   NOTE: the host MUST convert to ml_dtypes.bfloat16 -- feeding f32 numpy
   into a bf16 dram tensor silently reinterprets bytes on HW (NaNs).
 - 4 bf16 matmuls (full PE rate), N = 348 = 3 tau-blocks of
   [51 G-window | 64 U | 1 pad], one PSUM bank per g (bank-aligned).
 - the skew shift u is constant per 32-partition QUAD, so the time gather is
   16 plain 3-D copies at legal partition bases 0/32/64/96 on DVE/ACT
   reading PSUM directly (v1 burned the shared HWDGE ring on ~20 tiny DMAs;
   GPSIMD cannot touch PSUM, so Pool gets only SBUF work).
 - voice: U cols staged PSUM->SBUF (4 ACT copies), then 8 copy_predicated
   (per di) with mask mk[p, di] = ((p//4)%8 == di).  The mask is loaded and
   staged BEFORE the loop (u8 -> f32 on ACT -> u8 on DVE); the ACT op in
   that chain anchors the activation-table load on the loop-entry path,
   keeping 1.28us of LoadActFuncSet out of the For_i body.
 - output path is bf16 (tolerance 2e-2 >> measured 6.5e-3): halves store
   bytes and enables the DVE 2x 16-bit mode for the packed "+= vt" pass.
 - final add osb[p, tau*384+fj*8+r] = ts[p, tau*48+47-fj] + vt[..r] per
   tile: ACT/Pool broadcast-expand + DVE packed-2x add, or Pool/DVE fused
   tensor_add (ACT cannot tensor_add).
 - APs on shared tensors stay <= 3 dims and hot tensors are bufs=1:
   multi-buf tiles and 4-D APs degrade the dep tracker to whole-tensor-
   write, serializing every accessor pair (4-D is used only where the
   implied deps are real anyway: voice over usb/vt).
 - ts/osb are split per g so WAR deps retire per-tile in the loop.
 - out DRAM is PARTITION-MAJOR [t, p, m] so stores are 4 fully-contiguous
   3-tile DMAs (l-major order needs (k8,b,t,m) = 4 AP dims); the host
   un-permutes with one numpy transpose.  Stores split sync/scalar so the
   next iteration's load DMA is not queued behind them on SP's SEQ.
 - For_i(staggered_reset=True): stage-wise semaphore resets instead of a
   per-iteration all-engine barrier + drain (the plain For_i fully
   serializes iterations; with it the HW loop period was 24-27us).

Sharding: head-parallel, core h handles head h (4 batch rows of the output).
"""

import os
import sys

for _p in ("/opt/trn_rl_repo", "/root/.axon_site/_ro/trn_rl_repo"):
    if os.path.isdir(_p) and _p not in sys.path:
        sys.path.insert(0, _p)

import contextlib

import numpy as np

import concourse.bass as bass
import concourse.bacc as bacc
import concourse.mybir as mybir
import concourse.tile as tile
from concourse.bass_utils import run_bass_kernel_spmd

E, H, DI, DO, F = 16, 8, 8, 8, 48
L = F * DI  # 384
B = 4
NG = 51             # G window cols per tau-block
NU = DI * DO        # 64 voice cols
NBLK = NG + NU + 1  # 116 padded block width
NMM = 3 * NBLK      # 348: matmul N (even, >=256 for fp32r full rate)
GSTR = 512          # PSUM bank stride
NW = 4 * NMM        # 1392 W48 cols
NQW = 512 + NW      # 1904: fused [qt | W48] row length
NCORES = 8
DT = mybir.dt.float32
DTR = mybir.dt.float32r
DTH = mybir.dt.bfloat16

_prog_cache = {}

# mk[p, di] = 1 where (p//4) % 8 == di (voice copy_predicated select)
MSK = np.ascontiguousarray(
    ((np.arange(128)[:, None] // 4) % 8 == np.arange(8)[None, :]).astype(np.uint8)
)


def build_program(loop_n=None):
    nc = bacc.Bacc("TRN2", target_bir_lowering=False, debug=False)
    qw = nc.dram_tensor("qw", [48, NQW], DTH, kind="ExternalInput")
    mk = nc.dram_tensor("mk", [128, 8], mybir.dt.uint8, kind="ExternalInput")
    out = nc.dram_tensor("out", [12, 128, L], DTH, kind="ExternalOutput")

    with tile.TileContext(nc) as tc, contextlib.ExitStack() as ctx:
        const_pool = ctx.enter_context(tc.tile_pool(name="const", bufs=1))
        q_pool = ctx.enter_context(tc.tile_pool(name="q", bufs=2))
        zp_pool = ctx.enter_context(tc.tile_pool(name="zp", bufs=1, space="PSUM"))
        wk_pool = ctx.enter_context(tc.tile_pool(name="wk", bufs=1))

        # The voice mask is a kernel constant: load it BEFORE the loop, and
        # stage it through an ACT copy that voice reads.  The copy anchors an
        # activation op on the loop-entry path, which lets the act-table pass
        # keep the 1.28us LoadActFuncSet out of the For_i body.
        msk0 = const_pool.tile([128, 8], mybir.dt.uint8)
        nc.sync.dma_start(msk0[:], mk[:])
        mskf = const_pool.tile([128, 8], DT)
        nc.scalar.copy(mskf[:], msk0[:])     # also anchors the act table load
        msk = const_pool.tile([128, 8], mybir.dt.uint8)
        nc.vector.tensor_copy(msk[:], mskf[:])

        loop_ctx = (tc.For_i(0, loop_n, 1, staggered_reset=True)
                    if loop_n else contextlib.nullcontext())
        ctx.enter_context(loop_ctx)

        qws = q_pool.tile([48, NQW], DTH, tag="qws")
        nc.sync.dma_start(qws[:], qw[:])

        z_tiles = []
        for g in range(4):
            z_g = zp_pool.tile([128, GSTR], DT, tag=f"z{g}")
            nc.tensor.matmul(
                z_g[:, 0:NMM],
                qws[:, g * 128:(g + 1) * 128],
                qws[:, 512 + g * NMM:512 + (g + 1) * NMM],
            )
            z_tiles.append(z_g)

        # work tiles: ts/osb split per g (fine WAR retirement + precise deps);
        # usb/vt single so voice is 8 ops (their 4-D whole-tensor fallback
        # only creates dependencies that are real anyway)
        usb = wk_pool.tile([128, 768], DTH, tag="usb")
        tsg = [wk_pool.tile([128, 144], DTH, tag=f"ts{g}", name=f"ts{g}")
               for g in range(4)]
        vt = wk_pool.tile([128, 96], DTH, tag="vt")
        osb = [wk_pool.tile([128, 3 * L], DTH, tag=f"osb{j}", name=f"osb{j}")
               for j in range(4)]

        def u_copy(g):
            # usb[p, g*192 + tau*64 + 8di + do]  (ACT, from PSUM)
            u_src = bass.AP(
                z_tiles[g].tensor, NG, [[GSTR, 128], [NBLK, 3], [1, NU]]
            )
            u_dst = bass.AP(
                usb.tensor, g * 192, [[768, 128], [NU, 3], [1, NU]]
            )
            nc.scalar.copy(u_dst, u_src)

        def gather(u, g):
            # ts[g][p, tau*48 + k] = Z[p, tau*116 + u + k], 3-D from PSUM
            src = bass.AP(
                z_tiles[g].tensor, (32 * u) * GSTR + u,
                [[GSTR, 32], [NBLK, 3], [1, F]],
            )
            dst = bass.AP(
                tsg[g].tensor, (32 * u) * 144, [[144, 32], [F, 3], [1, F]]
            )
            if u % 2 == 1:
                nc.scalar.copy(dst, src)
            else:
                nc.vector.tensor_copy(dst, src)

        def voice(di):
            # vt[p, (3g+tau)*8 + r], predicated on di(p) == di (DVE)
            data = bass.AP(
                usb.tensor, 8 * di, [[768, 128], [192, 4], [1, 8], [NU, 3]]
            )
            mask = bass.AP(msk.tensor, di, [[8, 128], [0, 4], [0, 8], [0, 3]])
            vout = bass.AP(vt.tensor, 0, [[96, 128], [24, 4], [1, 8], [8, 3]])
            nc.vector.copy_predicated(vout, mask, data)

        def t_aps(g, tau):
            t_ap = bass.AP(
                tsg[g].tensor, tau * F + 47, [[144, 128], [-1, F], [0, 8]]
            )
            v_ap = bass.AP(
                vt.tensor, (3 * g + tau) * 8, [[96, 128], [0, F], [1, 8]]
            )
            o_ap = bass.AP(
                osb[g].tensor, tau * L, [[3 * L, 128], [8, F], [1, 8]]
            )
            return t_ap, v_ap, o_ap

        def expand(g, tau, eng):
            t_ap, _, o_ap = t_aps(g, tau)
            if eng is nc.scalar:
                nc.scalar.copy(o_ap, t_ap)
            else:
                nc.gpsimd.tensor_copy(o_ap, t_ap)

        def pass2(g, tau):
            _, v_ap, o_ap = t_aps(g, tau)
            nc.vector.tensor_add(o_ap, o_ap, v_ap)  # packed bf16 2x +=

        def fused(g, tau, eng=None):
            t_ap, v_ap, o_ap = t_aps(g, tau)
            (eng or nc.gpsimd).tensor_add(o_ap, t_ap, v_ap)

        def store(j):
            st_src = bass.AP(osb[j].tensor, 0, [[3 * L, 128], [L, 3], [1, L]])
            st_dst = bass.AP(
                out, j * 3 * 128 * L, [[L, 128], [128 * L, 3], [1, L]]
            )
            # j2/j3 on scalar: keeps SP's in-order SEQ free so the next
            # iteration's qw load issues right after j1
            (nc.sync if j < 2 else nc.scalar).dma_start(st_dst, st_src)

        u_copy(0), u_copy(1)
        for g in range(2):
            gather(0, g); gather(2, g)     # DVE
            gather(1, g); gather(3, g)     # ACT
        u_copy(2), u_copy(3)
        for g in range(2, 4):
            gather(0, g); gather(2, g)
            gather(1, g); gather(3, g)
        for di in range(8):
            voice(di)                      # DVE (needs all U)
        for g in range(4):
            expand(g, 0, nc.scalar if g != 2 else nc.gpsimd)
            expand(g, 1, nc.gpsimd if g < 2 else nc.scalar)
            pass2(g, 0)
            pass2(g, 1)
            fused(g, 2, eng=nc.vector if g == 3 else None)
            store(g)

    nc.compile()
    return nc


def _get_program():
    if "nc" not in _prog_cache:
        _prog_cache["nc"] = build_program()
    return _prog_cache["nc"]


def make_core_inputs(q, r_voice, e_past, e_future):
    """Host-side sharding: per-head [pretransposed-q | block-diag W48]."""
    q = np.ascontiguousarray(q, dtype=np.float32)
    qr = q.reshape(B, H, L, E)
    in_maps = []
    for h in range(NCORES):
        qh = qr[:, h]  # (B, L, E)
        # lhsT[16*tau + d, g*128 + p] = q[b, l, d],
        # p = 32u + 4di + b, l = 8*(12g + 4tau + u) + di
        lt = qh.reshape(B, 4, 3, 4, 8, E)      # (b, g, tau, u, di, d)
        lt = lt.transpose(2, 5, 1, 3, 4, 0)    # (tau, d, g, u, di, b)
        qw = np.zeros((48, NQW), dtype=np.float32)
        qw[:, :512] = lt.reshape(48, 4 * 128)
        master = np.zeros((E, 95), dtype=np.float32)
        master[:, :47] = e_future[1:48, :, h][::-1].T
        master[:, 47:] = e_past[:, :, h].T
        U = r_voice[:, :, :, h].reshape(DI * DO, E).T
        for g in range(4):
            for tau in range(3):
                c0 = 512 + g * NMM + tau * NBLK
                F0 = 4 * (3 * g + tau)
                qw[16 * tau:16 * tau + 16, c0:c0 + NG] = master[:, F0:F0 + NG]
                qw[16 * tau:16 * tau + 16, c0 + NG:c0 + NG + NU] = U
        qw16 = np.ascontiguousarray(qw.astype(mybir.dt.np(DTH)))
        in_maps.append({"qw": qw16, "mk": MSK})
    return in_maps


def kernel(q, flipped_masks, r_voice, e_past, e_future):
    q = np.asarray(q, dtype=np.float32)
    r_voice = np.asarray(r_voice, dtype=np.float32)
    e_past = np.asarray(e_past, dtype=np.float32)
    e_future = np.asarray(e_future, dtype=np.float32)

    nc = _get_program()
    in_maps = make_core_inputs(q, r_voice, e_past, e_future)
    res = run_bass_kernel_spmd(nc, in_maps, core_ids=list(range(NCORES)))

    # device out is [t, p, m] with p = 32u + 4di + b; un-permute to l-major
    # (l = 32t + 8u + di) and cast bf16 -> f32 host-side.
    out = np.empty((B * H, L, L), dtype=np.float32)
    for h in range(NCORES):
        arr = np.asarray(res.results[h]["out"], dtype=np.float32)
        arr = arr.reshape(12, 4, 8, B, L).transpose(3, 0, 1, 2, 4)  # (b,t,u,di,m)
        for b in range(B):
            out[b * H + h] = arr[b].reshape(L, L)
    return out
